# revision 9
# baseline (speedup 1.0000x reference)
"""GATNet (3-layer GAT, PyG-style) on 8 TRN2 NeuronCores.

Design: dst-sharded edge streaming in edge-partition layout.
- Table rows [h bf16(128) | a_s f32(4) | pad] = 512B, gathered by src via
  dma_gather (int16 idx, 4 address windows, <=1024 idx/call).
- a_d gathered by dst-local id (256B rows) via second dma_gather.
- Per 128-edge chunk: one-hot S (is_equal vs iota row) , wh = h*exp(lrelu(e)),
  matmul S^T.T@[wh|expe] accumulates [num|den] per 128-node block in PSUM,
  drained to SBUF accs. Softmax normalized post-hoc (no max subtraction).
- Between layers: AllGather of compact [h|a_s] u16 rows, repacked to 512B.
"""
import sys
sys.path.insert(0, "/opt/trn_rl_repo")
import numpy as np
import ml_dtypes

import concourse.bass as bass
import concourse.mybir as mybir
import concourse.tile as tile
import concourse.bacc as bacc
from concourse.bass_utils import run_bass_kernel_spmd
from concourse.library_config import mlp

P = 128
NCORES = 8
ROWU = 256          # u16 elems per padded table row (512B)
CROW = 136          # u16 elems per compact row: 128 bf16 h + 8 u16 (=4 f32 a_s)
ADROW = 64          # f32 elems per a_d table row (256B)
MAXC = 8            # chunks per dma_gather call (<=1024 idx)
LRELU = 0.2
EPS = 1e-5
H = 4
HID = 32

bf16 = ml_dtypes.bfloat16


def _f32_to_u16pair(a):
    """[..., k] f32 -> [..., 2k] u16 raw bytes."""
    return a.astype(np.float32).view(np.uint16)


def _pack_compact(h_bf16, a_s_f32):
    """[n,128] bf16 + [n,4] f32 -> [n, CROW] u16."""
    n = h_bf16.shape[0]
    out = np.zeros((n, CROW), dtype=np.uint16)
    out[:, :128] = h_bf16.view(np.uint16)
    out[:, 128:136] = _f32_to_u16pair(a_s_f32)
    return out


def _pad_rows(compact):
    n = compact.shape[0]
    out = np.zeros((n, ROWU), dtype=np.uint16)
    out[:, :CROW] = compact
    return out


def _wrap_idx(idx, ncols):
    """idx [n] int16 -> wrapped+replicated [128, ncols] int16."""
    out = np.zeros((P, ncols), dtype=np.int16)
    n16 = (len(idx) + 15) // 16
    w = np.zeros((16, n16), dtype=np.int16)
    pad = np.full(n16 * 16 - len(idx), -1, dtype=np.int16)
    full = np.concatenate([idx.astype(np.int16), pad])
    w = full.reshape(n16, 16).T
    for g in range(8):
        out[g * 16:(g + 1) * 16, :n16] = w
    return out


def _fuse_w(W, a_src, a_dst):
    """W [F,HC], a_src/a_dst [H,C] -> Wf [F, HC+8] with A_s, A_d block-diag."""
    F, HC = W.shape
    heads, C = a_src.shape
    A_s = np.zeros((HC, 4), dtype=np.float32)
    A_d = np.zeros((HC, 4), dtype=np.float32)
    for h in range(heads):
        A_s[h * C:(h + 1) * C, h] = a_src[h]
        A_d[h * C:(h + 1) * C, h] = a_dst[h]
    return np.concatenate([W, W @ A_s, W @ A_d], axis=1)  # [F, HC+8]


def _prep(x, edge_index):
    """Host preprocessing: sharding, windows, uniform schedule, index arrays."""
    N = x.shape[0]
    E = edge_index.shape[1]
    SHARD = ((N + NCORES * P - 1) // (NCORES * P)) * P
    NP_ = SHARD * NCORES
    NB = SHARD // P
    NW = 4
    WIN = ((NP_ + NW - 1) // NW + P - 1) // P * P
    assert WIN <= 32767

    loops = np.arange(N, dtype=np.int64)
    src = np.concatenate([edge_index[0].astype(np.int64), loops])
    dst = np.concatenate([edge_index[1].astype(np.int64), loops])

    core = dst // SHARD
    dstloc = dst % SHARD
    blk = dstloc // P
    loc_in_blk = dstloc % P
    w = np.minimum(src // WIN, NW - 1)
    src_rel = src - w * WIN

    # group counts per (core, w, blk)
    key = (core * NW + w) * NB + blk
    order = np.argsort(key, kind="stable")
    key_s = key[order]
    counts = np.bincount(key_s, minlength=NCORES * NW * NB).reshape(NCORES, NW, NB)
    chunks = (counts + P - 1) // P
    CH = chunks.max(axis=0)            # uniform per (w, blk)
    # calls per window
    sched = []                          # list of (w, C, [(blk, start, stop)])
    for wi in range(NW):
        stream = []                     # (blk, start, stop) per chunk
        for b in range(NB):
            for c in range(CH[wi, b]):
                stream.append((b, c == 0, c == CH[wi, b] - 1))
        for s in range(0, len(stream), MAXC):
            grp = stream[s:s + MAXC]
            sched.append((wi, len(grp), grp))
    ncalls = len(sched)
    tot_chunks = int(CH.sum())

    # per-core slot fill
    starts = np.zeros(NCORES * NW * NB + 1, dtype=np.int64)
    np.cumsum(np.bincount(key_s, minlength=NCORES * NW * NB), out=starts[1:])
    src16 = np.zeros((NCORES, P, 64 * ncalls), dtype=np.int16)
    dst16 = np.zeros((NCORES, P, 64 * ncalls), dtype=np.int16)
    dloc = np.full((NCORES, P, tot_chunks), 999.0, dtype=np.float32)

    src_rel_s = src_rel[order]
    loc_s = loc_in_blk[order]
    dstloc_s = dstloc[order]

    for ci in range(NCORES):
        ch_cursor = 0
        call_i = 0
        for wi in range(NW):
            # build the full slot arrays for this (core, window)
            nchunks_w = int(CH[wi].sum())
            s_slots = np.zeros(nchunks_w * P, dtype=np.int16)
            d_slots = np.zeros(nchunks_w * P, dtype=np.int16)
            l_slots = np.full(nchunks_w * P, 999.0, dtype=np.float32)
            off = 0
            for b in range(NB):
                k = (ci * NW + wi) * NB + b
                n = starts[k + 1] - starts[k]
                sl = slice(starts[k], starts[k + 1])
                s_slots[off:off + n] = src_rel_s[sl]
                d_slots[off:off + n] = dstloc_s[sl]
                l_slots[off:off + n] = loc_s[sl]
                off += CH[wi, b] * P
            # chunk-major layout: call covers chunks [c0, c0+C)
            c0 = 0
            while c0 < nchunks_w:
                C = min(MAXC, nchunks_w - c0)
                seg_s = s_slots[c0 * P:(c0 + C) * P]
                seg_d = d_slots[c0 * P:(c0 + C) * P]
                src16[ci, :, call_i * 64: call_i * 64 + (C * P) // 16] = _wrap_idx(
                    seg_s, (C * P) // 16)[:, :(C * P) // 16]
                dst16[ci, :, call_i * 64: call_i * 64 + (C * P) // 16] = _wrap_idx(
                    seg_d, (C * P) // 16)[:, :(C * P) // 16]
                lv = l_slots[c0 * P:(c0 + C) * P].reshape(C, P).T   # [P, C]
                dloc[ci, :, ch_cursor:ch_cursor + C] = lv
                ch_cursor += C
                call_i += 1
                c0 += C
        assert call_i == ncalls and ch_cursor == tot_chunks

    meta = dict(N=N, E=E, SHARD=SHARD, NP=NP_, NB=NB, NW=NW, WIN=WIN,
                sched=sched, ncalls=ncalls, tot_chunks=tot_chunks)
    return meta, src, dst, src16, dst16, dloc


def _build(meta):
    """Build the (uniform) 8-core Bass program."""
    SHARD, NB, WIN = meta["SHARD"], meta["NB"], meta["WIN"]
    NP_ = meta["NP"]
    sched = meta["sched"]
    ncalls = meta["ncalls"]
    tot_chunks = meta["tot_chunks"]

    nc = bacc.Bacc("TRN2", target_bir_lowering=False, debug=False,
                   num_devices=NCORES)
    dt = mybir.dt
    f32, u16, i16, bf = dt.float32, dt.uint16, dt.int16, dt.bfloat16

    t0pad = nc.declare_dram_parameter("t0pad", [NP_, ROWU], u16, isOutput=False)
    x_own = nc.declare_dram_parameter("x_own", [SHARD, P], f32, isOutput=False)
    adtab0 = nc.declare_dram_parameter("adtab0", [SHARD, ADROW], f32, isOutput=False)
    src16 = nc.declare_dram_parameter("src16", [P, 64 * ncalls], i16, isOutput=False)
    dst16 = nc.declare_dram_parameter("dst16", [P, 64 * ncalls], i16, isOutput=False)
    dloc = nc.declare_dram_parameter("dloc", [P, tot_chunks], bf, isOutput=False)
    iota_rep = nc.declare_dram_parameter("iota_rep", [P, P], bf, isOutput=False)
    ident = nc.declare_dram_parameter("ident", [P, P], f32, isOutput=False)
    wf1 = nc.declare_dram_parameter("wf1", [P, CROW], bf, isOutput=False)
    wf2 = nc.declare_dram_parameter("wf2", [P, CROW], bf, isOutput=False)
    lncons = nc.declare_dram_parameter("lncons", [P, P * 7], f32, isOutput=False)
    prel = nc.declare_dram_parameter("prel", [P, 4], f32, isOutput=False)
    out_ext = nc.declare_dram_parameter("out", [SHARD, P], f32, isOutput=True)

    with tile.TileContext(nc) as tc:
        with tc.tile_pool(name="cons", bufs=1) as cons, \
             tc.tile_pool(name="idxp", bufs=3) as idxp, \
             tc.tile_pool(name="slabp", bufs=3) as slabp, \
             tc.tile_pool(name="adp", bufs=3) as adp, \
             tc.tile_pool(name="whp", bufs=3) as whp, \
             tc.tile_pool(name="sp", bufs=3) as sp_, \
             tc.tile_pool(name="smallp", bufs=4) as smallp, \
             tc.tile_pool(name="accp", bufs=1) as accp, \
             tc.tile_pool(name="postp", bufs=4) as postp, \
             tc.tile_pool(name="psA", bufs=2, space="PSUM") as psA, \
             tc.tile_pool(name="psB", bufs=2, space="PSUM") as psB, \
             tc.tile_pool(name="dram", bufs=1, space="DRAM") as dram:

            nc.gpsimd.load_library(mlp)

            iota_t = cons.tile([P, P], bf)
            nc.sync.dma_start(out=iota_t[:], in_=iota_rep[:, :])
            ident_t = cons.tile([P, P], f32)
            nc.sync.dma_start(out=ident_t[:], in_=ident[:, :])
            wf_t = [cons.tile([P, CROW], bf, name=f"wft{i}", tag=f"wf{i}") for i in range(2)]
            nc.sync.dma_start(out=wf_t[0][:], in_=wf1[:, :])
            nc.sync.dma_start(out=wf_t[1][:], in_=wf2[:, :])
            lc = cons.tile([P, P * 7], f32)
            nc.sync.dma_start(out=lc[:], in_=lncons[:, :])
            # layout: [g0|be0|b0|g1|be1|b1|b2] each [P,P]
            pr = cons.tile([P, 4], f32)
            nc.sync.dma_start(out=pr[:], in_=prel[:, :])
            dloc_t = cons.tile([P, tot_chunks], bf)
            nc.sync.dma_start(out=dloc_t[:], in_=dloc[:, :])

            tpad = dram.tile([NP_, ROWU], u16)
            tsh = dram.tile([SHARD, CROW], u16)
            tfull = dram.tile([NP_, CROW], u16)
            adtab1 = dram.tile([SHARD, ADROW], f32)
            adtab2 = dram.tile([SHARD, ADROW], f32)
            xres = [dram.tile([SHARD, P], f32, name=f"xres{i}", tag=f"xres{i}") for i in range(2)]

            for layer in range(3):
                NH = 1 if layer == 2 else 4
                FH = P // NH
                tbl = t0pad if layer == 0 else tpad
                adt = adtab0 if layer == 0 else (adtab1 if layer == 1 else adtab2)

                accs = []
                for b in range(NB):
                    a = accp.tile([P, 132], f32, name=f"accb{b}", tag=f"acc{b}")
                    nc.vector.memset(a[:], 0.0)
                    accs.append(a)

                ch_cursor = 0
                ps_cur = None
                for call_i, (wi, C, grp) in enumerate(sched):
                    n16 = (C * P) // 16
                    sidx = idxp.tile([P, 64], i16, tag="sidx")
                    nc.sync.dma_start(out=sidx[:, :n16],
                                      in_=src16[:, call_i * 64: call_i * 64 + n16])
                    didx = idxp.tile([P, 64], i16, tag="didx")
                    nc.sync.dma_start(out=didx[:, :n16],
                                      in_=dst16[:, call_i * 64: call_i * 64 + n16])

                    slab = slabp.tile([P, MAXC * ROWU], u16, tag="slab")
                    nc.gpsimd.dma_gather(
                        out_ap=slab[:, : C * ROWU].rearrange(
                            "p (c e) -> p c e", e=ROWU),
                        in_ap=tbl[wi * WIN: min((wi + 1) * WIN, NP_), :],
                        idxs_ap=sidx[:, :n16],
                        num_idxs=C * P, num_idxs_reg=C * P,
                        elem_size=ROWU,
                    )
                    adsl = adp.tile([P, MAXC * ADROW], f32, tag="adsl")
                    nc.gpsimd.dma_gather(
                        out_ap=adsl[:, : C * ADROW].rearrange(
                            "p (c e) -> p c e", e=ADROW),
                        in_ap=adt[:, :],
                        idxs_ap=didx[:, :n16],
                        num_idxs=C * P, num_idxs_reg=C * P,
                        elem_size=ADROW,
                    )

                    # e = a_s + a_d ; lrelu; exp (f32), copy bf16
                    asv = slab[:, : C * ROWU].bitcast(f32).rearrange(
                        "p (c r) -> p c r", r=ROWU // 2)[:, :, 64:68]
                    adv = adsl[:, : C * ADROW].rearrange(
                        "p (c r) -> p c r", r=ADROW)[:, :, 0:4]
                    ee = smallp.tile([P, MAXC * 4], f32, tag="ee")
                    nc.vector.tensor_tensor(
                        out=ee[:, : C * 4].rearrange("p (c r) -> p c r", r=4),
                        in0=asv, in1=adv, op=mybir.AluOpType.add)
                    e2 = smallp.tile([P, MAXC * 4], f32, tag="e2")
                    nc.vector.tensor_scalar_mul(e2[:, : C * 4], ee[:, : C * 4], LRELU)
                    nc.vector.tensor_tensor(out=ee[:, : C * 4], in0=ee[:, : C * 4],
                                            in1=e2[:, : C * 4], op=mybir.AluOpType.max)
                    nc.scalar.activation(out=ee[:, : C * 4], in_=ee[:, : C * 4],
                                         func=mybir.ActivationFunctionType.Exp)
                    eb = smallp.tile([P, MAXC * 4], bf, tag="eb")
                    nc.vector.tensor_copy(out=eb[:, : C * 4], in_=ee[:, : C * 4])

                    # S slab: is_equal(dstloc, iota)
                    Ss = sp_.tile([P, MAXC * P], bf, tag="Ss")
                    dv = dloc_t[:, ch_cursor:ch_cursor + C]
                    nc.vector.tensor_tensor(
                        out=Ss[:, : C * P].rearrange("p (c f) -> p c f", f=P),
                        in0=dv.unsqueeze(2).to_broadcast([P, C, P]),
                        in1=iota_t[:].unsqueeze(1).to_broadcast([P, C, P]),
                        op=mybir.AluOpType.is_equal)

                    # wh slab: [wh(128) | expe(4)] per chunk, bf16
                    wh = whp.tile([P, MAXC * 132], bf, tag="wh")
                    hbv = slab[:, : C * ROWU].bitcast(bf).rearrange(
                        "p (c r) -> p c r", r=ROWU)[:, :, 0:128].rearrange(
                        "p c (h f) -> p c h f", f=FH)
                    ebv = eb[:, : C * 4].rearrange("p (c h) -> p c h", h=4)
                    whv = wh[:, : C * 132].rearrange(
                        "p (c r) -> p c r", r=132)
                    nc.vector.tensor_tensor(
                        out=whv[:, :, 0:128].rearrange(
                            "p c (h f) -> p c h f", f=FH),
                        in0=hbv,
                        in1=ebv[:, :, 0:NH].unsqueeze(3).to_broadcast([P, C, NH, FH]),
                        op=mybir.AluOpType.mult)
                    nc.vector.tensor_copy(out=whv[:, :, 128:132],
                                          in_=ebv)

                    for c, (b, st, sp2) in enumerate(grp):
                        if st:
                            ps_cur = psA.tile([P, 132], f32, tag="ps")
                        nc.tensor.matmul(
                            ps_cur[:],
                            Ss[:, c * P:(c + 1) * P],
                            wh[:, c * 132:(c + 1) * 132],
                            start=st, stop=sp2)
                        if sp2:
                            nc.vector.tensor_add(accs[b][:], accs[b][:], ps_cur[:])
                    ch_cursor += C

                # post per block
                for b in range(NB):
                    acc = accs[b]
                    den = postp.tile([P, 4], f32, tag="den")
                    nc.vector.tensor_scalar_add(den[:], acc[:, 128:132], 1e-16)
                    rden = postp.tile([P, 4], f32, tag="rden")
                    nc.vector.reciprocal(rden[:], den[:])
                    o = postp.tile([P, P], f32, tag="o")
                    nc.vector.tensor_tensor(
                        out=o[:].rearrange("p (h f) -> p h f", f=FH),
                        in0=acc[:, 0:128].rearrange("p (h f) -> p h f", f=FH),
                        in1=rden[:, 0:NH].unsqueeze(2).to_broadcast([P, NH, FH]),
                        op=mybir.AluOpType.mult)

                    if layer < 2:
                        gofs, beofs, bofs = layer * 3 * P, (layer * 3 + 1) * P, (layer * 3 + 2) * P
                        nc.vector.tensor_add(o[:], o[:], lc[:, bofs:bofs + P])
                        # LayerNorm
                        mu = postp.tile([P, 1], f32, tag="mu")
                        nc.vector.tensor_reduce(mu[:], o[:], axis=mybir.AxisListType.X,
                                                op=mybir.AluOpType.add)
                        nc.vector.tensor_scalar_mul(mu[:], mu[:], 1.0 / P)
                        d_ = postp.tile([P, P], f32, tag="d_")
                        nc.vector.tensor_scalar_sub(d_[:], o[:], mu[:])
                        sq = postp.tile([P, P], f32, tag="sq")
                        nc.vector.tensor_tensor(out=sq[:], in0=d_[:], in1=d_[:],
                                                op=mybir.AluOpType.mult)
                        var = postp.tile([P, 1], f32, tag="var")
                        nc.vector.tensor_reduce(var[:], sq[:], axis=mybir.AxisListType.X,
                                                op=mybir.AluOpType.add)
                        nc.vector.tensor_scalar_mul(var[:], var[:], 1.0 / P)
                        sd = postp.tile([P, 1], f32, tag="sd")
                        nc.scalar.activation(out=sd[:], in_=var[:],
                                             func=mybir.ActivationFunctionType.Sqrt,
                                             bias=pr[:, 2:3])
                        rsd = postp.tile([P, 1], f32, tag="rsd")
                        nc.vector.reciprocal(rsd[:], sd[:])
                        nc.vector.tensor_scalar_mul(d_[:], d_[:], rsd[:])
                        nc.vector.tensor_tensor(out=d_[:], in0=d_[:],
                                                in1=lc[:, gofs:gofs + P],
                                                op=mybir.AluOpType.mult)
                        nc.vector.tensor_add(d_[:], d_[:], lc[:, beofs:beofs + P])
                        # PReLU: r + p*(x-r)
                        r_ = postp.tile([P, P], f32, tag="r_")
                        nc.scalar.activation(out=r_[:], in_=d_[:],
                                             func=mybir.ActivationFunctionType.Relu)
                        nc.vector.tensor_tensor(out=d_[:], in0=d_[:], in1=r_[:],
                                                op=mybir.AluOpType.subtract)
                        nc.vector.tensor_scalar_mul(d_[:], d_[:],
                                                    pr[:, layer:layer + 1])
                        nc.vector.tensor_add(d_[:], d_[:], r_[:])
                        # residual
                        xr = postp.tile([P, P], f32, tag="xr")
                        rsrc = x_own if layer == 0 else xres[0]
                        nc.sync.dma_start(out=xr[:], in_=rsrc[b * P:(b + 1) * P, :])
                        nc.vector.tensor_add(d_[:], d_[:], xr[:])
                        # store residual for next layer
                        wdst = xres[0] if layer == 0 else xres[1]
                        nc.sync.dma_start(out=wdst[b * P:(b + 1) * P, :], in_=d_[:])
                        # table build: T = d_ @ Wf
                        tps = psB.tile([P, P], f32, tag="tps")
                        nc.tensor.transpose(tps[:], d_[:], ident_t[:])
                        xT = postp.tile([P, P], bf, tag="xT")
                        nc.vector.tensor_copy(out=xT[:], in_=tps[:])
                        tps2 = psB.tile([P, CROW], f32, tag="tps2")
                        nc.tensor.matmul(tps2[:], xT[:], wf_t[layer][:],
                                         start=True, stop=True)
                        pk = postp.tile([P, CROW], u16, tag="pk")
                        nc.vector.tensor_copy(out=pk[:, 0:128].bitcast(bf),
                                              in_=tps2[:, 0:128])
                        nc.vector.tensor_copy(out=pk[:, 128:136].bitcast(f32),
                                              in_=tps2[:, 128:132])
                        nc.sync.dma_start(out=tsh[b * P:(b + 1) * P, :], in_=pk[:])
                        adw = postp.tile([P, 4], f32, tag="adw")
                        nc.vector.tensor_copy(out=adw[:], in_=tps2[:, 132:136])
                        adn = adtab1 if layer == 0 else adtab2
                        nc.sync.dma_start(
                            out=adn[b * P:(b + 1) * P, 0:4], in_=adw[:])
                    else:
                        nc.vector.tensor_add(o[:], o[:], lc[:, 6 * P:7 * P])
                        xr = postp.tile([P, P], f32, tag="xr")
                        nc.sync.dma_start(out=xr[:], in_=xres[1][b * P:(b + 1) * P, :])
                        nc.vector.tensor_add(o[:], o[:], xr[:])
                        nc.sync.dma_start(out=out_ext[b * P:(b + 1) * P, :], in_=o[:])

                if layer < 2:
                    nc.gpsimd.collective_compute(
                        "AllGather", mybir.AluOpType.bypass,
                        replica_groups=[list(range(NCORES))],
                        ins=[tsh.opt()], outs=[tfull.opt()])
                    # repack compact -> padded 512B rows
                    for t in range(NP_ // P):
                        rp = postp.tile([P, ROWU], u16, tag="rp")
                        nc.sync.dma_start(out=rp[:, :CROW],
                                          in_=tfull[t * P:(t + 1) * P, :])
                        nc.sync.dma_start(out=tpad[t * P:(t + 1) * P, :], in_=rp[:])

    nc.compile()
    return nc


def kernel(x, edge_index, W0, a_src0, a_dst0, b0, g0, be0, p0,
           W1, a_src1, a_dst1, b1, g1, be1, p1,
           W2, a_src2, a_dst2, b2):
    x = np.asarray(x, dtype=np.float32)
    edge_index = np.asarray(edge_index)
    meta, src, dst, src16, dst16, dloc = _prep(x, edge_index)
    N, SHARD, NP_ = meta["N"], meta["SHARD"], meta["NP"]

    # host: layer-0 table
    Wf0 = _fuse_w(np.asarray(W0, np.float32), np.asarray(a_src0, np.float32),
                  np.asarray(a_dst0, np.float32))
    xp = np.zeros((NP_, P), dtype=np.float32)
    xp[:N] = x
    T0 = xp @ Wf0                       # [NP, 136]
    t0c = _pack_compact(T0[:, 0:128].astype(bf16), T0[:, 128:132])
    t0pad = _pad_rows(t0c)
    adfull = np.zeros((NP_, ADROW), dtype=np.float32)
    adfull[:, 0:4] = T0[:, 132:136]

    Wf1 = _fuse_w(np.asarray(W1, np.float32), np.asarray(a_src1, np.float32),
                  np.asarray(a_dst1, np.float32))
    Wf2f = _fuse_w(np.asarray(W2, np.float32), np.asarray(a_src2, np.float32),
                   np.asarray(a_dst2, np.float32))  # [128, 130]
    wf1a = np.zeros((P, CROW), dtype=bf16)
    wf1a[:, :136] = Wf1.astype(bf16)
    wf2a = np.zeros((P, CROW), dtype=bf16)
    wf2a[:, :136] = Wf2f.astype(bf16)          # [W2 | A_s(4) | A_d(4)]

    iota = np.tile(np.arange(P, dtype=np.float32)[None, :], (P, 1)).astype(bf16)
    ident = np.eye(P, dtype=np.float32)
    lncons = np.zeros((P, P * 7), dtype=np.float32)
    for i, v in enumerate([g0, be0, b0, g1, be1, b1, b2]):
        lncons[:, i * P:(i + 1) * P] = np.tile(np.asarray(v, np.float32)[None, :], (P, 1))
    prel = np.zeros((P, 4), dtype=np.float32)
    prel[:, 0] = float(np.asarray(p0).reshape(-1)[0])
    prel[:, 1] = float(np.asarray(p1).reshape(-1)[0])
    prel[:, 2] = EPS

    nc = _build(meta)

    in_maps = []
    for ci in range(NCORES):
        in_maps.append(dict(
            t0pad=t0pad,
            x_own=xp[ci * SHARD:(ci + 1) * SHARD],
            adtab0=adfull[ci * SHARD:(ci + 1) * SHARD],
            src16=src16[ci], dst16=dst16[ci],
            dloc=dloc[ci].astype(bf16),
            iota_rep=iota, ident=ident,
            wf1=wf1a, wf2=wf2a, lncons=lncons, prel=prel,
        ))
    import os
    iters = int(os.environ.get("GAT_TIME_ITERS", "0"))
    if iters <= 0:
        res = run_bass_kernel_spmd(nc, in_maps, core_ids=list(range(NCORES)))
        outs = [res.results[ci]["out"] for ci in range(NCORES)]
    else:
        outs = _run_timed(nc, in_maps, iters)
    out = np.concatenate(outs, axis=0)
    return out[:N].astype(np.float32)


LAST_EXEC_NS = -1


def _run_timed(nc, in_maps, iters):
    """Mirror bass2jax.run_bass_via_pjrt multi-core path, but keep inputs on
    device and run `iters` pipelined executions to estimate per-run time."""
    global LAST_EXEC_NS
    import time
    import jax
    from jax.sharding import Mesh, PartitionSpec
    from jax.experimental.shard_map import shard_map
    from concourse import bass2jax as b2j
    from concourse import mybir as mb

    b2j.install_neuronx_cc_hook()
    n_cores = len(in_maps)
    partition_name = nc.partition_id_tensor.name if nc.partition_id_tensor else None
    in_names, out_names, out_avals, zero_outs = [], [], [], []
    for alloc in nc.m.functions[0].allocations:
        if not isinstance(mb.MemoryLocationSet, type) or not isinstance(alloc, mb.MemoryLocationSet):
            continue
        assert alloc.memorylocations
        name = alloc.memorylocations[0].name
        if alloc.kind == "ExternalInput":
            if name != partition_name:
                in_names.append(name)
        elif alloc.kind == "ExternalOutput":
            shp = list(alloc.tensor_shape)
            dtp = mb.dt.np(alloc.dtype)
            out_names.append(name)
            out_avals.append(jax.core.ShapedArray(tuple(shp), dtp))
            zero_outs.append(np.zeros(shp, dtp))
    n_params = len(in_names)
    in_names = in_names + out_names
    if partition_name is not None:
        in_names.append(partition_name)

    def _body(*args):
        operands = list(args)
        if partition_name is not None:
            operands.append(b2j.partition_id_tensor())
        return tuple(b2j._bass_exec_p.bind(
            *operands, out_avals=tuple(out_avals), in_names=tuple(in_names),
            out_names=tuple(out_names), lowering_input_output_aliases=(),
            sim_require_finite=True, sim_require_nnan=True, nc=nc))

    devices = jax.devices()[:n_cores]
    mesh = Mesh(np.asarray(devices), ("core",))
    nin = n_params + len(out_names)
    sharded = jax.jit(
        shard_map(_body, mesh=mesh, in_specs=(PartitionSpec("core"),) * nin,
                  out_specs=(PartitionSpec("core"),) * len(out_names),
                  check_rep=False),
        keep_unused=True)
    per_core = [[np.asarray(m[k]) for k in in_names[:n_params]] for m in in_maps]
    concat_in = [np.concatenate([per_core[c][i] for c in range(n_cores)], axis=0)
                 for i in range(n_params)]
    concat_zeros = [np.zeros((n_cores * z.shape[0], *z.shape[1:]), z.dtype)
                    for z in zero_outs]
    from jax.sharding import NamedSharding
    shard = NamedSharding(mesh, PartitionSpec("core"))
    dev_in = [jax.device_put(a, shard) for a in concat_in + concat_zeros]
    o = sharded(*dev_in)
    jax.block_until_ready(o)
    t0 = time.time()
    for _ in range(iters):
        o = sharded(*dev_in)
    jax.block_until_ready(o)
    dt_ = (time.time() - t0) / iters
    LAST_EXEC_NS = int(dt_ * 1e9)
    arrs = [np.asarray(o[i]).reshape(n_cores, *out_avals[i].shape)
            for i in range(len(out_names))]
    return [arrs[0][c] for c in range(n_cores)]


if __name__ == "__main__":
    pass
